# revision 1
# baseline (speedup 1.0000x reference)
"""LinOSS layer Trainium2 kernel.

Math: the per-state 2x2 recurrence matrix M = [[1, -sA], [s, 1-s^2 A]] has
det(M)=1 and eigenvalues e^{+-i theta} with cos(theta) = 1 - s^2 A / 2, so
M^d = p_d M - p_{d-1} I with p_d = sin(d theta)/sin(theta).  The scanned state
x_t collapses to a rank-2 modulated prefix sum:

    u_t   = s * Bu_t            (s folded into B on host)
    T1    = gamma*cos(t th) + sin(t th);  T2 = cos(t th) - gamma*sin(t th)
    E     = cumsum(T1 * u);     F = cumsum(T2 * u)
    x_t   = sin(t th) * E_t + cos(t th) * F_t
    gamma = (s - s^2 A / 2) / sin(theta)

Sharding: states P=256 split across 8 cores (32 each); inside a core, time
L=8192 is folded 4x into partitions -> tiles are (128=[4 chunks x 32 states],
2048).  Fold-chunk carries are fixed with per-partition cumsum offsets
(strictly-lower chunk mask matmul).  Each core emits a partial (H, L) output
(its 32-state slice of ys @ C^T, plus input*D on core 0 only); the host sums
partials and transposes - that is the unshard/all-reduce step for this
sharding.
"""

import numpy as np

L, H, P = 8192, 128, 256
NCORES = 8
SLOC = P // NCORES          # states per core
FOLD = 4                    # time chunks folded into partitions
CL = L // FOLD              # 2048 free columns per partition row
NPART = FOLD * SLOC         # 128
SEED = 128                  # host-seeded table width
DOUBLINGS = [128, 256, 512, 1024]
JT = 512                    # j-tile width (psum bank)
NJT = CL // JT              # 4
NTT = L // 128              # 64 transpose tiles

_CACHE: dict = {}


def _build_bass(split_waits=True):
    import concourse.bass as bass
    import concourse.mybir as mybir
    import concourse.tile as tile
    from concourse.masks import make_identity

    dt = mybir.dt.float32
    bt = mybir.dt.bfloat16
    Alu = mybir.AluOpType

    nc = bass.Bass(
        trn_type="TRN2",
        target_bir_lowering=False,
        debug=False,
        num_devices=NCORES,
    )

    inp = nc.dram_tensor("inp", [L, H], bt, kind="ExternalInput").ap()
    Bt_d = nc.dram_tensor("Bt", [H, 2 * SLOC], bt, kind="ExternalInput").ap()
    Ctr_d = nc.dram_tensor("Ctr", [NPART, H], bt, kind="ExternalInput").ap()
    Cti_d = nc.dram_tensor("Cti", [NPART, H], bt, kind="ExternalInput").ap()
    dD_d = nc.dram_tensor("dD", [H, H], bt, kind="ExternalInput").ap()
    Wm_d = nc.dram_tensor("Wm", [NPART, NPART], dt, kind="ExternalInput").ap()
    consts_d = nc.dram_tensor("consts", [NPART, 16], dt, kind="ExternalInput").ap()
    seedS_d = nc.dram_tensor("seedS", [NPART, SEED], bt, kind="ExternalInput").ap()
    seedC_d = nc.dram_tensor("seedC", [NPART, SEED], bt, kind="ExternalInput").ap()
    outp = nc.dram_tensor("outp", [H, L], dt, kind="ExternalOutput").ap()

    with tile.TileContext(nc) as tc:
        cpool = tc.alloc_tile_pool(name="const", bufs=1)
        big1 = tc.alloc_tile_pool(name="big1", bufs=1)
        work = tc.alloc_tile_pool(name="work", bufs=2)
        evac = tc.alloc_tile_pool(name="evac", bufs=2)
        psum = tc.alloc_tile_pool(name="psum", bufs=2, space="PSUM")
        psum_bu = tc.alloc_tile_pool(name="psum_bu", bufs=2, space="PSUM")
        big2 = tc.alloc_tile_pool(name="big2", bufs=1)

        consts = cpool.tile_from(consts_d)
        inpT = big1.tile([128, L], bt, tag="inpT")
        for q in range(FOLD):
            nc.sync.dma_start_transpose(
                out=inpT[:, q * CL : (q + 1) * CL],
                in_=inp[q * CL : (q + 1) * CL, :],
            )
        Bt = cpool.tile_from(Bt_d)
        Ctr = cpool.tile_from(Ctr_d)
        Cti = cpool.tile_from(Cti_d)
        dD = cpool.tile_from(dD_d)
        Wm = cpool.tile_from(Wm_d)
        ones = cpool.tile([NPART, CL], dt)
        nc.vector.memset(ones[:], 1.0)


        gamma = consts[:, 0:1]
        gamma_neg = consts[:, 1:2]
        cosD = [consts[:, 2 + m : 3 + m] for m in range(4)]
        sinD = [consts[:, 6 + m : 7 + m] for m in range(4)]

        # ---- sin/cos tables (global angles), built by angle-doubling ----
        sinT = big1.tile([NPART, CL], bt, tag="sinT")
        cosT = big1.tile([NPART, CL], bt, tag="cosT")
        nc.sync.dma_start(out=sinT[:, 0:SEED], in_=seedS_d)
        nc.sync.dma_start(out=cosT[:, 0:SEED], in_=seedC_d)
        n = SEED
        for m, nn_ in enumerate(DOUBLINGS):
            assert nn_ == n
            t1 = work.tile([NPART, n], bt, tag="tbl")
            t2 = work.tile([NPART, n], bt, tag="tbl")
            # sin(x+D) = sin x cos D + cos x sin D
            nc.scalar.activation(
                t1[:], cosT[:, 0:n], mybir.ActivationFunctionType.Copy,
                scale=sinD[m],
            )
            nc.vector.scalar_tensor_tensor(
                sinT[:, n : 2 * n], sinT[:, 0:n], cosD[m], t1[:],
                Alu.mult, Alu.add,
            )
            # cos(x+D) = cos x cos D - sin x sin D
            nc.scalar.activation(
                t2[:], sinT[:, 0:n], mybir.ActivationFunctionType.Copy,
                scale=sinD[m],
            )
            nc.vector.scalar_tensor_tensor(
                cosT[:, n : 2 * n], cosT[:, 0:n], cosD[m], t2[:],
                Alu.mult, Alu.subtract,
            )
            n *= 2
        assert n == CL

        # ---- input load (tiled) + on-chip transpose to (H x L) ----
        T1 = big2.tile([NPART, CL], bt, tag="T1")
        T2 = big2.tile([NPART, CL], bt, tag="T2")
        nc.vector.scalar_tensor_tensor(
            T1[:], cosT[:], gamma, sinT[:], Alu.mult, Alu.add
        )
        nc.vector.scalar_tensor_tensor(
            T2[:], sinT[:], gamma_neg, cosT[:], Alu.mult, Alu.add
        )

        # ---- Bu matmuls + modulation + chained scans ----
        Er = big2.tile([NPART, CL], dt, tag="Er")
        Fr = big2.tile([NPART, CL], dt, tag="Fr")
        Ei = big2.tile([NPART, CL], dt, tag="Ei")
        Fi = big2.tile([NPART, CL], dt, tag="Fi")
        EFs = [Er, Fr, Ei, Fi]
        Y1r = big2.tile([NPART, CL], dt, tag="Y1r")
        Y2r = big2.tile([NPART, CL], dt, tag="Y2r")
        Y1i = big2.tile([NPART, CL], dt, tag="Y1i")
        Y2i = big2.tile([NPART, CL], dt, tag="Y2i")


        for jt in range(NJT):
            js = slice(jt * JT, (jt + 1) * JT)
            pbu_r = psum_bu.tile([NPART, JT], dt, tag="bu_r")
            pbu_i = psum_bu.tile([NPART, JT], dt, tag="bu_i")
            for c in range(FOLD):
                rhs = inpT[:, c * CL + jt * JT : c * CL + (jt + 1) * JT]
                ps = slice(c * SLOC, (c + 1) * SLOC)
                nc.tensor.matmul(
                    pbu_r[ps, :], Bt[:, 0:SLOC], rhs, start=True, stop=True,
                    tile_position=(0, c * SLOC),
                )
                nc.tensor.matmul(
                    pbu_i[ps, :], Bt[:, SLOC : 2 * SLOC], rhs,
                    start=True, stop=True,
                    tile_position=(0, c * SLOC),
                )
            u_r = evac.tile([NPART, JT], bt, tag="u_r")
            u_i = evac.tile([NPART, JT], bt, tag="u_i")
            nc.scalar.copy(u_r[:], pbu_r[:])
            nc.scalar.copy(u_i[:], pbu_i[:])
            nc.vector.tensor_mul(Y1r[:, js], u_r[:], T1[:, js])
            nc.gpsimd.tensor_mul(Y2r[:, js], u_r[:], T2[:, js])
            nc.vector.tensor_mul(Y1i[:, js], u_i[:], T1[:, js])
            nc.gpsimd.tensor_mul(Y2i[:, js], u_i[:], T2[:, js])

        for arr, y in zip(EFs, [Y1r, Y2r, Y1i, Y2i]):
            # builder lives on BassGpSimd, but TRN2 runs the scan on DVE
            bass.BassGpSimd.tensor_tensor_scan(
                nc.vector, arr[:], ones[:], y[:], 0.0, Alu.mult, Alu.add
            )

        # ---- fold-chunk carry offsets ----
        fins = cpool.tile([NPART, 4], dt)
        for i, arr in enumerate(EFs):
            nc.scalar.copy(fins[:, i : i + 1], arr[:, CL - 1 : CL])
        poff = psum.tile([NPART, 4], dt, tag="out")
        nc.tensor.matmul(poff[:], Wm[:], fins[:], start=True, stop=True)
        offs = cpool.tile([NPART, 4], dt)
        nc.scalar.copy(offs[:], poff[:])

        # ---- demodulate + project + D-term + store ----
        for jt in range(NJT):
            js = slice(jt * JT, (jt + 1) * JT)
            eEr = work.tile([NPART, JT], bt, tag="w0")
            eFr = work.tile([NPART, JT], bt, tag="w1")
            eEi = work.tile([NPART, JT], bt, tag="w2")
            eFi = work.tile([NPART, JT], bt, tag="w3")
            Ident = mybir.ActivationFunctionType.Identity
            nc.scalar.activation(eEr[:], Er[:, js], Ident, bias=offs[:, 0:1])
            nc.scalar.activation(eFr[:], Fr[:, js], Ident, bias=offs[:, 1:2])
            nc.scalar.activation(eEi[:], Ei[:, js], Ident, bias=offs[:, 2:3])
            nc.scalar.activation(eFi[:], Fi[:, js], Ident, bias=offs[:, 3:4])
            t1r = work.tile([NPART, JT], bt, tag="w4")
            t2r = work.tile([NPART, JT], bt, tag="w5")
            t1i = work.tile([NPART, JT], bt, tag="w6")
            t2i = work.tile([NPART, JT], bt, tag="w7")
            x_r = work.tile([NPART, JT], bt, tag="w8")
            x_i = work.tile([NPART, JT], bt, tag="w9")
            nc.vector.tensor_mul(t1r[:], eEr[:], sinT[:, js])
            nc.gpsimd.tensor_mul(t2r[:], eFr[:], cosT[:, js])
            nc.vector.tensor_mul(t1i[:], eEi[:], sinT[:, js])
            nc.gpsimd.tensor_mul(t2i[:], eFi[:], cosT[:, js])
            nc.vector.tensor_add(x_r[:], t1r[:], t2r[:])
            nc.gpsimd.tensor_add(x_i[:], t1i[:], t2i[:])
            for c in range(FOLD):
                ps = slice(c * SLOC, (c + 1) * SLOC)
                po = psum.tile([128, JT], dt, tag="out")
                nc.tensor.matmul(
                    po[:], Ctr[ps, :], x_r[ps, :], start=True, stop=False,
                    tile_position=(c * SLOC, 0),
                )
                nc.tensor.matmul(
                    po[:], Cti[ps, :], x_i[ps, :],
                    start=False, stop=False,
                    tile_position=(c * SLOC, 0),
                )
                nc.tensor.matmul(
                    po[:], dD[:],
                    inpT[:, c * CL + jt * JT : c * CL + (jt + 1) * JT],
                    start=False, stop=True,
                )
                osb = evac.tile([128, JT], dt, tag="osb")
                nc.scalar.copy(osb[:], po[:])
                nc.sync.dma_start(
                    out=outp[:, c * CL + jt * JT : c * CL + (jt + 1) * JT],
                    in_=osb[:],
                )
        for p in (big2, psum_bu, psum, evac, work, big1, cpool):
            p.release()
    if split_waits:
        _split_matmul_waits(nc, mybir)
    return nc


def _split_matmul_waits(nc, mybir):
    """Hardware instruction structs fit a limited number of embedded sync
    waits (1 for the fp32 self-loading LDWEIGHTS matmul, 2 for ACT/DVE/POOL
    compute structs); move extra waits onto an inserted same-queue no-op."""
    caps = {"InstMatmult": 1}
    skip = {"InstNoOp", "InstAllEngineBarrier", "InstSync"}
    k = 0
    for bb in nc.main_func.blocks:
        insts = bb.instructions
        i = 0
        while i < len(insts):
            ins = insts[i]
            tn = type(ins).__name__
            if tn not in skip and ins.sync_info is not None:
                cap = caps.get(tn, 1)
                w = list(ins.sync_info.on_wait or [])
                if len(w) > cap:
                    for wj in w[:-cap]:
                        nop = mybir.InstNoOp(
                            name=f"I-mmdep-{k}",
                            engine=ins.engine,
                            ins=[],
                            outs=[],
                            sync_info=mybir.SyncInfo(
                                on_wait=[wj], on_update=[]
                            ),
                        )
                        k += 1
                        insts.insert(i, nop)
                        i += 1
                    ins.sync_info = mybir.SyncInfo(
                        on_wait=w[-cap:], on_update=ins.sync_info.on_update
                    )
            i += 1


def _host_prep(inputs):
    import ml_dtypes
    inp = np.ascontiguousarray(
        np.asarray(inputs["input_sequence"], np.float32).astype(ml_dtypes.bfloat16)
    )
    A = np.maximum(np.asarray(inputs["A_diag_raw"], np.float64), 0.0)
    s = 1.0 / (1.0 + np.exp(-np.asarray(inputs["steps_raw"], np.float64)))
    Br = np.asarray(inputs["B_real"], np.float64)
    Bi = np.asarray(inputs["B_img"], np.float64)
    Cr = np.asarray(inputs["C_real"], np.float64)
    Ci = np.asarray(inputs["C_img"], np.float64)
    D = np.asarray(inputs["D"], np.float64)

    costh = 1.0 - s * s * A / 2.0
    sinth = np.sqrt(np.maximum(1.0 - costh * costh, 1e-300))
    theta = np.arctan2(sinth, costh)
    gamma = (s - s * s * A / 2.0) / sinth

    import ml_dtypes
    f32 = np.float32
    bf16 = ml_dtypes.bfloat16
    in_maps = []
    twopi = 2.0 * np.pi
    for k in range(NCORES):
        sl = slice(k * SLOC, (k + 1) * SLOC)
        th = theta[sl]  # (SLOC,)
        Bt = np.empty((H, 2 * SLOC), bf16)
        Bt[:, 0:SLOC] = (s[sl, None] * Br[sl]).T.astype(bf16)
        Bt[:, SLOC:] = (s[sl, None] * Bi[sl]).T.astype(bf16)
        Ctr = np.tile(Cr[:, sl].T, (FOLD, 1)).astype(bf16)
        Cti = np.tile(-Ci[:, sl].T, (FOLD, 1)).astype(bf16)
        dD = (np.diag(D) if k == 0 else np.zeros((H, H))).astype(bf16)

        # per-partition q = c*SLOC + s
        th_q = np.tile(th, FOLD)  # (NPART,)
        tbase = np.repeat(np.arange(FOLD) * CL, SLOC).astype(np.float64)
        consts = np.zeros((NPART, 16), f32)
        consts[:, 0] = np.tile(gamma[sl], FOLD)
        consts[:, 1] = -consts[:, 0]
        for m, n in enumerate(DOUBLINGS):
            ang = np.mod(n * th_q, twopi)
            consts[:, 2 + m] = np.cos(ang)
            consts[:, 6 + m] = np.sin(ang)
        j = np.arange(SEED, dtype=np.float64)
        ang0 = np.mod((tbase[:, None] + j[None, :]) * th_q[:, None], twopi)
        seedS = np.sin(ang0).astype(bf16)
        seedC = np.cos(ang0).astype(bf16)

        q = np.arange(NPART)
        Wm = ((q[:, None] % SLOC == q[None, :] % SLOC)
              & (q[:, None] // SLOC < q[None, :] // SLOC)).astype(f32)

        in_maps.append({
            "inp": inp,
            "Bt": Bt,
            "Ctr": Ctr,
            "Cti": Cti,
            "dD": dD,
            "Wm": Wm,
            "consts": consts,
            "seedS": seedS,
            "seedC": seedC,
        })
    return in_maps


LAST_RESULTS = None


def kernel(**inputs) -> np.ndarray:
    global LAST_RESULTS
    from concourse.bass_utils import run_bass_kernel_spmd

    if "nc" not in _CACHE:
        _CACHE["nc"] = _build_bass()
    nc = _CACHE["nc"]

    in_maps = _host_prep(inputs)
    res = run_bass_kernel_spmd(nc, in_maps, core_ids=list(range(NCORES)))
    LAST_RESULTS = res
    part = np.zeros((H, L), np.float32)
    for r in res.results:
        part += r["outp"]
    return np.ascontiguousarray(part.T)



# revision 7
# speedup vs baseline: 1.2376x; 1.2376x over previous
"""LinOSS layer Trainium2 kernel.

Math: the per-state 2x2 recurrence matrix M = [[1, -sA], [s, 1-s^2 A]] has
det(M)=1 and eigenvalues e^{+-i theta} with cos(theta) = 1 - s^2 A / 2, so the
scanned state collapses to a rank-2 modulated prefix sum:

    u_t   = s * Bu_t            (s folded into B on host)
    T1    = gamma*cos(t th) + sin(t th);  T2 = cos(t th) - gamma*sin(t th)
    E     = cumsum(T1 * u);     F = cumsum(T2 * u)
    x_t   = sin(t th) * E_t + cos(t th) * F_t
    gamma = (s - s^2 A / 2) / sin(theta)

Sharding: states P=256 split across 8 cores (32 each); inside a core, time
L=8192 is folded 4x into partitions -> tiles are (128=[4 chunks x 32 states],
2048).  Fold-chunk carries are fixed with per-partition offsets computed from
row sums (accum_out of the modulation ops) via a strictly-lower chunk mask
matmul, folded into the demodulation as a per-partition bias.  Each core emits
a partial (H, L) bf16 output (its 32-state slice of ys @ C^T); the host sums
partials, adds input*D, and transposes - the unshard/all-reduce step.

All tables (sin/cos/T1/T2) are host-precomputed in fp64 and DMAed as bf16;
everything elementwise on-chip is bf16 (DVE 2x mode), scans carry fp32 state
internally per the ISA.
"""

import numpy as np

L, H, P = 8192, 128, 256
NCORES = 8
SLOC = P // NCORES          # states per core
FOLD = 4                    # time chunks folded into partitions
CL = L // FOLD              # 2048 free columns per partition row
NPART = FOLD * SLOC         # 128
JT = 512                    # j-tile width (psum bank)
NJT = CL // JT              # 4
SCH = 1024                  # scan chunk (2 j-tiles)

_CACHE: dict = {}


def _build_bass(split_waits=True):
    import concourse.bass as bass
    import concourse.mybir as mybir
    import concourse.tile as tile

    dt = mybir.dt.float32
    bt = mybir.dt.bfloat16
    Alu = mybir.AluOpType

    nc = bass.Bass(
        trn_type="TRN2",
        target_bir_lowering=False,
        debug=False,
        num_devices=NCORES,
    )

    inp = nc.dram_tensor("inp", [L, H], bt, kind="ExternalInput").ap()
    Bt_d = nc.dram_tensor("Bt", [H, 2 * SLOC], bt, kind="ExternalInput").ap()
    T1_d = nc.dram_tensor("T1", [NPART, CL], bt, kind="ExternalInput").ap()
    T2_d = nc.dram_tensor("T2", [NPART, CL], bt, kind="ExternalInput").ap()
    Sd_d = nc.dram_tensor("Sd", [NPART, CL], bt, kind="ExternalInput").ap()
    Cd_d = nc.dram_tensor("Cd", [NPART, CL], bt, kind="ExternalInput").ap()
    Ctr_d = nc.dram_tensor("Ctr", [NPART, H], bt, kind="ExternalInput").ap()
    Cti_d = nc.dram_tensor("Cti", [NPART, H], bt, kind="ExternalInput").ap()
    Wm_d = nc.dram_tensor("Wm", [NPART, NPART], bt, kind="ExternalInput").ap()
    outp = nc.dram_tensor("outp", [H, L], bt, kind="ExternalOutput").ap()

    with tile.TileContext(nc) as tc:
        cpool = tc.alloc_tile_pool(name="const", bufs=1)
        big = tc.alloc_tile_pool(name="big", bufs=1)
        evac = tc.alloc_tile_pool(name="evac", bufs=2)
        osbp = tc.alloc_tile_pool(name="osbp", bufs=2)
        psum_bu = tc.alloc_tile_pool(name="psum_bu", bufs=2, space="PSUM")
        psum_o = tc.alloc_tile_pool(name="psum_o", bufs=4, space="PSUM")

        # priority DMAs: weights + modulation tables first
        Bt = cpool.tile_from(Bt_d)
        T1 = big.tile_from(T1_d)
        T2 = big.tile_from(T2_d)

        # input transpose, 16 pieces in jt-major order so the Bu pipeline can
        # start early; Act issues jt0, SP the rest (both are HWDGE engines)
        inpT = big.tile([NPART, L], bt, tag="inpT")
        for jt in range(NJT):
            eng = nc.scalar if jt == 0 else nc.sync
            for c in range(FOLD):
                lo = c * CL + jt * JT
                eng.dma_start_transpose(
                    out=inpT[:, lo : lo + JT], in_=inp[lo : lo + JT, :]
                )

        Sd = big.tile_from(Sd_d)
        Cd = big.tile_from(Cd_d)
        Ctr = cpool.tile_from(Ctr_d)
        Cti = cpool.tile_from(Cti_d)
        Wm = cpool.tile_from(Wm_d)

        ones = cpool.tile([NPART, CL], bt)
        nc.gpsimd.memset(ones[:], 1.0)

        Y1r = big.tile([NPART, CL], bt, tag="Y1r")
        Y2r = big.tile([NPART, CL], bt, tag="Y2r")
        Y1i = big.tile([NPART, CL], bt, tag="Y1i")
        Y2i = big.tile([NPART, CL], bt, tag="Y2i")
        Er = big.tile([NPART, CL], bt, tag="Er")
        Fr = big.tile([NPART, CL], bt, tag="Fr")
        Ei = big.tile([NPART, CL], bt, tag="Ei")
        Fi = big.tile([NPART, CL], bt, tag="Fi")

        # ---- Bu matmuls + modulation (+row-sum accum) + chunked scans ----
        for jt in range(NJT):
            js = slice(jt * JT, (jt + 1) * JT)
            pbu_r = psum_bu.tile([NPART, JT], dt, tag="bu_r")
            pbu_i = psum_bu.tile([NPART, JT], dt, tag="bu_i")
            for c in range(FOLD):
                rhs = inpT[:, c * CL + jt * JT : c * CL + (jt + 1) * JT]
                ps = slice(c * SLOC, (c + 1) * SLOC)
                nc.tensor.matmul(
                    pbu_r[ps, :], Bt[:, 0:SLOC], rhs, start=True, stop=True,
                    tile_position=(0, c * SLOC),
                )
                nc.tensor.matmul(
                    pbu_i[ps, :], Bt[:, SLOC : 2 * SLOC], rhs,
                    start=True, stop=True,
                    tile_position=(0, c * SLOC),
                )
            u_r = evac.tile([NPART, JT], bt, tag="u_r")
            u_i = evac.tile([NPART, JT], bt, tag="u_i")
            nc.scalar.copy(u_r[:], pbu_r[:])
            nc.scalar.copy(u_i[:], pbu_i[:])
            nc.vector.tensor_mul(Y1r[:, js], u_r[:], T1[:, js])
            nc.vector.tensor_mul(Y2r[:, js], u_r[:], T2[:, js])
            nc.vector.tensor_mul(Y1i[:, js], u_i[:], T1[:, js])
            nc.gpsimd.tensor_mul(Y2i[:, js], u_i[:], T2[:, js])
            if jt % 2 == 1:
                ch = jt // 2
                sc = slice(ch * SCH, (ch + 1) * SCH)
                for arr, y in (
                    (Er, Y1r), (Fr, Y2r), (Ei, Y1i), (Fi, Y2i),
                ):
                    init = 0.0 if ch == 0 else arr[:, ch * SCH - 1 : ch * SCH]
                    bass.BassGpSimd.tensor_tensor_scan(
                        nc.vector, arr[:, sc], ones[:, sc], y[:, sc], init,
                        Alu.mult, Alu.add,
                    )

        # ---- fold-chunk carry offsets from the scan finals ----
        fins = cpool.tile([NPART, 4], bt)
        for i, arr in enumerate((Er, Fr, Ei, Fi)):
            nc.scalar.copy(fins[:, i : i + 1], arr[:, CL - 1 : CL])
        poff = psum_o.tile([NPART, 4], dt, tag="out")
        nc.tensor.matmul(poff[:], Wm[:], fins[:], start=True, stop=True)
        offs = cpool.tile([NPART, 4], dt)
        nc.scalar.copy(offs[:], poff[:])

        # ---- demodulate (bias fused) + project + store, per scan chunk ----
        t1r = big.tile([NPART, CL], bt, tag="t1r")
        t2r = big.tile([NPART, CL], bt, tag="t2r")
        t1i = big.tile([NPART, CL], bt, tag="t1i")
        t2i = big.tile([NPART, CL], bt, tag="t2i")
        x_r = big.tile([NPART, CL], bt, tag="x_r")
        x_i = big.tile([NPART, CL], bt, tag="x_i")
        eFr = big.tile([NPART, CL], bt, tag="eFr")
        eFi = big.tile([NPART, CL], bt, tag="eFi")
        Ident = mybir.ActivationFunctionType.Identity
        for ch in range(CL // SCH):
            sc = slice(ch * SCH, (ch + 1) * SCH)
            nc.vector.scalar_tensor_tensor(
                t1r[:, sc], Er[:, sc], offs[:, 0:1], Sd[:, sc], Alu.add, Alu.mult
            )
            nc.scalar.activation(eFr[:, sc], Fr[:, sc], Ident, bias=offs[:, 1:2])
            nc.gpsimd.tensor_mul(t2r[:, sc], eFr[:, sc], Cd[:, sc])
            nc.vector.scalar_tensor_tensor(
                t1i[:, sc], Ei[:, sc], offs[:, 2:3], Sd[:, sc], Alu.add, Alu.mult
            )
            nc.scalar.activation(eFi[:, sc], Fi[:, sc], Ident, bias=offs[:, 3:4])
            nc.gpsimd.tensor_mul(t2i[:, sc], eFi[:, sc], Cd[:, sc])
            nc.vector.tensor_add(x_r[:, sc], t1r[:, sc], t2r[:, sc])
            nc.gpsimd.tensor_add(x_i[:, sc], t1i[:, sc], t2i[:, sc])
            for jt in range(2 * ch, 2 * ch + 2):
                js = slice(jt * JT, (jt + 1) * JT)
                for c in range(FOLD):
                    ps = slice(c * SLOC, (c + 1) * SLOC)
                    po = psum_o.tile([NPART, JT], dt, tag="out")
                    nc.tensor.matmul(
                        po[:], Ctr[ps, :], x_r[ps, js], start=True, stop=False,
                        tile_position=(c * SLOC, 0),
                    )
                    nc.tensor.matmul(
                        po[:], Cti[ps, :], x_i[ps, js], start=False, stop=True,
                        tile_position=(c * SLOC, 0),
                    )
                    k = jt * FOLD + c
                    osb = osbp.tile([NPART, JT], bt, tag=f"osb{k % 4}")
                    if k % 4 < 2:
                        nc.scalar.copy(osb[:], po[:])
                    else:
                        nc.vector.tensor_scalar_add(osb[:], po[:], 0.0)
                    nc.sync.dma_start(
                        out=outp[:, c * CL + jt * JT : c * CL + (jt + 1) * JT],
                        in_=osb[:],
                    )
        for p in (psum_o, psum_bu, osbp, evac, big, cpool):
            p.release()
    if split_waits:
        _split_matmul_waits(nc, mybir)
    return nc


def _split_matmul_waits(nc, mybir):
    """Hardware instruction structs fit a limited number of embedded sync
    waits (1 for the fp32 self-loading LDWEIGHTS matmul, 2 for ACT/DVE/POOL
    compute structs); move extra waits onto an inserted same-queue no-op."""
    caps = {"InstMatmult": 1}
    skip = {"InstNoOp", "InstAllEngineBarrier", "InstSync"}
    k = 0
    for bb in nc.main_func.blocks:
        insts = bb.instructions
        i = 0
        while i < len(insts):
            ins = insts[i]
            tn = type(ins).__name__
            if tn not in skip and ins.sync_info is not None:
                cap = caps.get(tn, 1)
                w = list(ins.sync_info.on_wait or [])
                if len(w) > cap:
                    for wj in w[:-cap]:
                        nop = mybir.InstNoOp(
                            name=f"I-mmdep-{k}",
                            engine=ins.engine,
                            ins=[],
                            outs=[],
                            sync_info=mybir.SyncInfo(
                                on_wait=[wj], on_update=[]
                            ),
                        )
                        k += 1
                        insts.insert(i, nop)
                        i += 1
                    ins.sync_info = mybir.SyncInfo(
                        on_wait=w[-cap:], on_update=ins.sync_info.on_update
                    )
            i += 1


def _host_prep(inputs):
    import ml_dtypes
    bf16 = ml_dtypes.bfloat16
    f32 = np.float32
    inp = np.ascontiguousarray(
        np.asarray(inputs["input_sequence"], f32).astype(bf16)
    )
    A = np.maximum(np.asarray(inputs["A_diag_raw"], np.float64), 0.0)
    s = 1.0 / (1.0 + np.exp(-np.asarray(inputs["steps_raw"], np.float64)))
    Br = np.asarray(inputs["B_real"], np.float64)
    Bi = np.asarray(inputs["B_img"], np.float64)
    Cr = np.asarray(inputs["C_real"], np.float64)
    Ci = np.asarray(inputs["C_img"], np.float64)

    costh = 1.0 - s * s * A / 2.0
    sinth = np.sqrt(np.maximum(1.0 - costh * costh, 1e-300))
    theta = np.arctan2(sinth, costh)
    gamma = (s - s * s * A / 2.0) / sinth

    twopi = 2.0 * np.pi
    q = np.arange(NPART)
    Wm_f = ((q[:, None] % SLOC == q[None, :] % SLOC)
            & (q[:, None] // SLOC < q[None, :] // SLOC)).astype(bf16)
    tbase = np.repeat(np.arange(FOLD) * CL, SLOC).astype(np.float64)
    j = np.arange(CL, dtype=np.float64)

    in_maps = []
    for k in range(NCORES):
        sl = slice(k * SLOC, (k + 1) * SLOC)
        Bt = np.empty((H, 2 * SLOC), bf16)
        Bt[:, 0:SLOC] = (s[sl, None] * Br[sl]).T.astype(bf16)
        Bt[:, SLOC:] = (s[sl, None] * Bi[sl]).T.astype(bf16)
        Ctr = np.tile(Cr[:, sl].T, (FOLD, 1)).astype(bf16)
        Cti = np.tile(-Ci[:, sl].T, (FOLD, 1)).astype(bf16)

        th_q = np.tile(theta[sl], FOLD)          # (NPART,)
        g_q = np.tile(gamma[sl], FOLD)[:, None]  # (NPART, 1)
        ang = np.mod((tbase[:, None] + j[None, :]) * th_q[:, None], twopi)
        sinT = np.sin(ang)
        cosT = np.cos(ang)
        in_maps.append({
            "inp": inp,
            "Bt": Bt,
            "T1": (g_q * cosT + sinT).astype(bf16),
            "T2": (cosT - g_q * sinT).astype(bf16),
            "Sd": sinT.astype(bf16),
            "Cd": cosT.astype(bf16),
            "Ctr": Ctr,
            "Cti": Cti,
            "Wm": Wm_f,
        })
    return in_maps


LAST_RESULTS = None


def kernel(**inputs) -> np.ndarray:
    global LAST_RESULTS
    from concourse.bass_utils import run_bass_kernel_spmd

    if "nc" not in _CACHE:
        _CACHE["nc"] = _build_bass()
    nc = _CACHE["nc"]

    in_maps = _host_prep(inputs)
    res = run_bass_kernel_spmd(nc, in_maps, core_ids=list(range(NCORES)))
    LAST_RESULTS = res
    part = np.zeros((H, L), np.float32)
    for r in res.results:
        part += r["outp"].astype(np.float32)
    out = part.T + np.asarray(inputs["input_sequence"], np.float32) * np.asarray(
        inputs["D"], np.float32
    )
    return np.ascontiguousarray(out)


# revision 8
# speedup vs baseline: 1.3757x; 1.1116x over previous
"""LinOSS layer Trainium2 kernel.

Math: the per-state 2x2 recurrence matrix M = [[1, -sA], [s, 1-s^2 A]] has
det(M)=1 and eigenvalues e^{+-i theta} with cos(theta) = 1 - s^2 A / 2, so the
scanned state collapses to a rank-2 modulated prefix sum:

    u_t   = s * Bu_t            (s folded into B on host)
    T1    = gamma*cos(t th) + sin(t th);  T2 = cos(t th) - gamma*sin(t th)
    E     = cumsum(T1 * u);     F = cumsum(T2 * u)
    x_t   = sin(t th) * E_t + cos(t th) * F_t
    gamma = (s - s^2 A / 2) / sin(theta)

Sharding: states P=256 split across 8 cores (32 each); inside a core, time
L=8192 is folded 4x into partitions -> tiles are (128=[4 chunks x 32 states],
2048).  Fold-chunk carries are fixed with per-partition offsets (scan finals
through a strictly-lower chunk-mask matmul) folded into the demodulation as a
per-partition bias.  Each core emits a partial (H, L) bf16 output (its
32-state slice of ys @ C^T); the host sums partials, adds input*D, and
transposes - the unshard/all-reduce step.

Device-side structure per core:
  - input arrives HOST-pretransposed in jt-major layout (no DMA transpose)
  - real/imag are processed PAIRED (one [128, 2, 512] op via strided APs)
  - the DVE scan runs at 2 cycles/col regardless of dtype -> it is the
    critical resource; everything else is pushed to PE/Act/Pool
  - x = t1 + t2 is absorbed into the projection via PSUM accumulation
  - all tables are host-precomputed fp64 -> bf16
"""

import numpy as np

L, H, P = 8192, 128, 256
NCORES = 8
SLOC = P // NCORES          # states per core
FOLD = 4                    # time chunks folded into partitions
CL = L // FOLD              # 2048 free columns per partition row
NPART = FOLD * SLOC         # 128
JT = 512                    # j-tile width (psum bank)
NJT = CL // JT              # 4
SCH = 1024                  # scan chunk (2 j-tiles)
CL2 = 2 * CL

_CACHE: dict = {}


def _build_bass(split_waits=True):
    import concourse.bass as bass
    import concourse.mybir as mybir
    import concourse.tile as tile

    dt = mybir.dt.float32
    bt = mybir.dt.bfloat16
    Alu = mybir.AluOpType

    nc = bass.Bass(
        trn_type="TRN2",
        target_bir_lowering=False,
        debug=False,
        num_devices=NCORES,
    )

    inpT_d = nc.dram_tensor("inpT", [NPART, L], bt, kind="ExternalInput").ap()
    Bt_d = nc.dram_tensor("Bt", [H, 2 * SLOC], bt, kind="ExternalInput").ap()
    T1_d = nc.dram_tensor("T1d", [NPART, CL2], bt, kind="ExternalInput").ap()
    T2_d = nc.dram_tensor("T2d", [NPART, CL2], bt, kind="ExternalInput").ap()
    Sd_d = nc.dram_tensor("Sd", [NPART, CL], bt, kind="ExternalInput").ap()
    Cdd_d = nc.dram_tensor("Cdd", [NPART, CL2], bt, kind="ExternalInput").ap()
    Ctr_d = nc.dram_tensor("Ctr", [NPART, H], bt, kind="ExternalInput").ap()
    Cti_d = nc.dram_tensor("Cti", [NPART, H], bt, kind="ExternalInput").ap()
    Wm_d = nc.dram_tensor("Wm", [NPART, NPART], bt, kind="ExternalInput").ap()
    outp = nc.dram_tensor("outp", [H, L], bt, kind="ExternalOutput").ap()

    def pair(ap, width):
        # [128, 2*CL] -> [128, 2, width] strided view (r half | i half)
        return ap.rearrange("p (two cl) -> p two cl", two=2)[:, :, 0:width]

    def pairsl(ap, sl):
        return ap.rearrange("p (two cl) -> p two cl", two=2)[:, :, sl]

    with tile.TileContext(nc) as tc:
        cpool = tc.alloc_tile_pool(name="const", bufs=1)
        big = tc.alloc_tile_pool(name="big", bufs=1)
        evac = tc.alloc_tile_pool(name="evac", bufs=2)
        osbp = tc.alloc_tile_pool(name="osbp", bufs=2)
        psum_bu = tc.alloc_tile_pool(name="psum_bu", bufs=2, space="PSUM")
        psum_o = tc.alloc_tile_pool(name="psum_o", bufs=2, space="PSUM")

        # DMA priority order: Bu weights, first input block, mod tables, rest
        Bt = cpool.tile_from(Bt_d)
        inpT = big.tile([NPART, L], bt, tag="inpT")
        nc.sync.dma_start(out=inpT[:, 0:2048], in_=inpT_d[:, 0:2048])
        T1 = big.tile_from(T1_d)
        nc.sync.dma_start(out=inpT[:, 2048:4096], in_=inpT_d[:, 2048:4096])
        T2 = big.tile_from(T2_d)
        nc.sync.dma_start(out=inpT[:, 4096:6144], in_=inpT_d[:, 4096:6144])
        nc.sync.dma_start(out=inpT[:, 6144:8192], in_=inpT_d[:, 6144:8192])
        Sd = big.tile_from(Sd_d)
        Cdd = big.tile_from(Cdd_d)
        Ctr = cpool.tile_from(Ctr_d)
        Cti = cpool.tile_from(Cti_d)
        Wm = cpool.tile_from(Wm_d)

        ones = cpool.tile([NPART, SCH], bt)
        nc.gpsimd.memset(ones[:], 1.0)

        Y1 = big.tile([NPART, CL2], bt, tag="Y1")   # (T1*u_r | T1*u_i)
        Y2 = big.tile([NPART, CL2], bt, tag="Y2")   # (T2*u_r | T2*u_i)
        E1 = big.tile([NPART, CL2], bt, tag="E1")   # (Er | Ei)
        E2 = big.tile([NPART, CL2], bt, tag="E2")   # (Fr | Fi)

        # ---- Bu matmuls + paired modulation + chunked scans ----
        for jt in range(NJT):
            pbu = psum_bu.tile([NPART, 2 * JT], dt, tag="bu")
            for c in range(FOLD):
                rhs = inpT[:, jt * CL + c * JT : jt * CL + (c + 1) * JT]
                ps = slice(c * SLOC, (c + 1) * SLOC)
                nc.tensor.matmul(
                    pbu[ps, 0:JT], Bt[:, 0:SLOC], rhs, start=True, stop=True,
                    tile_position=(0, c * SLOC),
                )
                nc.tensor.matmul(
                    pbu[ps, JT : 2 * JT], Bt[:, SLOC : 2 * SLOC], rhs,
                    start=True, stop=True,
                    tile_position=(0, c * SLOC),
                )
            U = evac.tile([NPART, 2 * JT], bt, tag="U")
            nc.scalar.copy(U[:], pbu[:])
            js = slice(jt * JT, (jt + 1) * JT)
            Uv = U[:].rearrange("p (two j) -> p two j", two=2)
            nc.vector.tensor_mul(pairsl(Y1[:], js), Uv, pairsl(T1[:], js))
            nc.gpsimd.tensor_mul(pairsl(Y2[:], js), Uv, pairsl(T2[:], js))
            if jt % 2 == 1:
                ch = jt // 2
                sc = slice(ch * SCH, (ch + 1) * SCH)
                sci = slice(CL + ch * SCH, CL + (ch + 1) * SCH)
                for arr, y in ((E1, Y1), (E2, Y2)):
                    for s in (sc, sci):
                        init = (
                            0.0 if ch == 0
                            else arr[:, s.start - 1 : s.start]
                        )
                        bass.BassGpSimd.tensor_tensor_scan(
                            nc.vector, arr[:, s], ones[:], y[:, s], init,
                            Alu.mult, Alu.add,
                        )

        # ---- fold-chunk carry offsets from the scan finals ----
        fins = cpool.tile([NPART, 4], bt)
        for i, (arr, col) in enumerate(
            ((E1, CL), (E2, CL), (E1, CL2), (E2, CL2))
        ):
            nc.scalar.copy(fins[:, i : i + 1], arr[:, col - 1 : col])
        poff = psum_o.tile([NPART, 4], dt, tag="out")
        nc.tensor.matmul(poff[:], Wm[:], fins[:], start=True, stop=True)
        offs = cpool.tile([NPART, 4], dt)
        nc.scalar.copy(offs[:], poff[:])

        # ---- demodulate (bias fused) + project (sum in PSUM) + store ----
        t1 = big.tile([NPART, CL2], bt, tag="t1")
        t2 = big.tile([NPART, CL2], bt, tag="t2")
        eF = big.tile([NPART, CL2], bt, tag="eF")
        Ident = mybir.ActivationFunctionType.Identity
        for ch in range(CL // SCH):
            sc = slice(ch * SCH, (ch + 1) * SCH)
            sci = slice(CL + ch * SCH, CL + (ch + 1) * SCH)
            # t1 = (E + offE) * sin  (stt, fused bias)
            nc.vector.scalar_tensor_tensor(
                t1[:, sc], E1[:, sc], offs[:, 0:1], Sd[:, sc],
                Alu.add, Alu.mult,
            )
            nc.vector.scalar_tensor_tensor(
                t1[:, sci], E1[:, sci], offs[:, 2:3], Sd[:, sc],
                Alu.add, Alu.mult,
            )
            # t2 = (F + offF) * cos  (Act bias, then paired Pool/V muls)
            nc.scalar.activation(eF[:, sc], E2[:, sc], Ident, bias=offs[:, 1:2])
            nc.scalar.activation(eF[:, sci], E2[:, sci], Ident, bias=offs[:, 3:4])
            nc.gpsimd.tensor_mul(t2[:, sc], eF[:, sc], Cdd[:, sc])
            nc.vector.tensor_mul(t2[:, sci], eF[:, sci], Cdd[:, sc])
            for c in range(FOLD):
                ps = slice(c * SLOC, (c + 1) * SLOC)
                po = psum_o.tile([NPART, 2 * JT], dt, tag="out")
                for half in range(2):
                    js = slice((2 * ch + half) * JT, (2 * ch + half + 1) * JT)
                    jsi = slice(CL + js.start, CL + js.stop)
                    pd = slice(half * JT, (half + 1) * JT)
                    nc.tensor.matmul(
                        po[:, pd], Ctr[ps, :], t1[ps, js],
                        start=True, stop=False, tile_position=(c * SLOC, 0),
                    )
                    nc.tensor.matmul(
                        po[:, pd], Ctr[ps, :], t2[ps, js],
                        start=False, stop=False, tile_position=(c * SLOC, 0),
                    )
                    nc.tensor.matmul(
                        po[:, pd], Cti[ps, :], t1[ps, jsi],
                        start=False, stop=False, tile_position=(c * SLOC, 0),
                    )
                    nc.tensor.matmul(
                        po[:, pd], Cti[ps, :], t2[ps, jsi],
                        start=False, stop=True, tile_position=(c * SLOC, 0),
                    )
                osb = osbp.tile([NPART, 2 * JT], bt, tag="osb")
                nc.scalar.copy(osb[:], po[:])
                nc.sync.dma_start(
                    out=outp[:, c * CL + 2 * ch * JT : c * CL + (2 * ch + 2) * JT],
                    in_=osb[:],
                )
        for p in (psum_o, psum_bu, osbp, evac, big, cpool):
            p.release()
    if split_waits:
        _split_matmul_waits(nc, mybir)
    return nc


def _split_matmul_waits(nc, mybir):
    """Hardware instruction structs fit a limited number of embedded sync
    waits (1 for the fp32 self-loading LDWEIGHTS matmul, 2 for ACT/DVE/POOL
    compute structs); move extra waits onto an inserted same-queue no-op."""
    caps = {"InstMatmult": 1}
    skip = {"InstNoOp", "InstAllEngineBarrier", "InstSync"}
    k = 0
    for bb in nc.main_func.blocks:
        insts = bb.instructions
        i = 0
        while i < len(insts):
            ins = insts[i]
            tn = type(ins).__name__
            if tn not in skip and ins.sync_info is not None:
                cap = caps.get(tn, 1)
                w = list(ins.sync_info.on_wait or [])
                if len(w) > cap:
                    for wj in w[:-cap]:
                        nop = mybir.InstNoOp(
                            name=f"I-mmdep-{k}",
                            engine=ins.engine,
                            ins=[],
                            outs=[],
                            sync_info=mybir.SyncInfo(
                                on_wait=[wj], on_update=[]
                            ),
                        )
                        k += 1
                        insts.insert(i, nop)
                        i += 1
                    ins.sync_info = mybir.SyncInfo(
                        on_wait=w[-cap:], on_update=ins.sync_info.on_update
                    )
            i += 1


def _host_prep(inputs):
    import ml_dtypes
    bf16 = ml_dtypes.bfloat16
    inp = np.asarray(inputs["input_sequence"], np.float32)
    # host pre-transpose into jt-major layout: SBUF col jt*CL + c*JT + j
    # maps natural time t = c*CL + jt*JT + j
    inpT = np.ascontiguousarray(
        inp.T.reshape(H, FOLD, NJT, JT).transpose(0, 2, 1, 3).reshape(H, L)
    ).astype(bf16)
    A = np.maximum(np.asarray(inputs["A_diag_raw"], np.float64), 0.0)
    s = 1.0 / (1.0 + np.exp(-np.asarray(inputs["steps_raw"], np.float64)))
    Br = np.asarray(inputs["B_real"], np.float64)
    Bi = np.asarray(inputs["B_img"], np.float64)
    Cr = np.asarray(inputs["C_real"], np.float64)
    Ci = np.asarray(inputs["C_img"], np.float64)

    costh = 1.0 - s * s * A / 2.0
    sinth = np.sqrt(np.maximum(1.0 - costh * costh, 1e-300))
    theta = np.arctan2(sinth, costh)
    gamma = (s - s * s * A / 2.0) / sinth

    twopi = 2.0 * np.pi
    q = np.arange(NPART)
    Wm_f = ((q[:, None] % SLOC == q[None, :] % SLOC)
            & (q[:, None] // SLOC < q[None, :] // SLOC)).astype(bf16)
    tbase = np.repeat(np.arange(FOLD) * CL, SLOC).astype(np.float64)
    j = np.arange(CL, dtype=np.float64)

    in_maps = []
    for k in range(NCORES):
        sl = slice(k * SLOC, (k + 1) * SLOC)
        Bt = np.empty((H, 2 * SLOC), bf16)
        Bt[:, 0:SLOC] = (s[sl, None] * Br[sl]).T.astype(bf16)
        Bt[:, SLOC:] = (s[sl, None] * Bi[sl]).T.astype(bf16)
        Ctr = np.tile(Cr[:, sl].T, (FOLD, 1)).astype(bf16)
        Cti = np.tile(-Ci[:, sl].T, (FOLD, 1)).astype(bf16)

        th_q = np.tile(theta[sl], FOLD)          # (NPART,)
        g_q = np.tile(gamma[sl], FOLD)[:, None]  # (NPART, 1)
        ang = np.mod((tbase[:, None] + j[None, :]) * th_q[:, None], twopi)
        sinT = np.sin(ang)
        cosT = np.cos(ang)
        T1 = (g_q * cosT + sinT).astype(bf16)
        T2 = (cosT - g_q * sinT).astype(bf16)
        Cdb = cosT.astype(bf16)
        in_maps.append({
            "inpT": inpT,
            "Bt": Bt,
            "T1d": np.concatenate([T1, T1], axis=1),
            "T2d": np.concatenate([T2, T2], axis=1),
            "Sd": sinT.astype(bf16),
            "Cdd": np.concatenate([Cdb, Cdb], axis=1),
            "Ctr": Ctr,
            "Cti": Cti,
            "Wm": Wm_f,
        })
    return in_maps


LAST_RESULTS = None


def kernel(**inputs) -> np.ndarray:
    global LAST_RESULTS
    from concourse.bass_utils import run_bass_kernel_spmd

    if "nc" not in _CACHE:
        _CACHE["nc"] = _build_bass()
    nc = _CACHE["nc"]

    in_maps = _host_prep(inputs)
    res = run_bass_kernel_spmd(nc, in_maps, core_ids=list(range(NCORES)))
    LAST_RESULTS = res
    part = np.zeros((H, L), np.float32)
    for r in res.results:
        part += r["outp"].astype(np.float32)
    out = part.T + np.asarray(inputs["input_sequence"], np.float32) * np.asarray(
        inputs["D"], np.float32
    )
    return np.ascontiguousarray(out)


# revision 12
# speedup vs baseline: 1.4930x; 1.0853x over previous
"""LinOSS layer Trainium2 kernel.

Math: the per-state 2x2 recurrence matrix M = [[1, -sA], [s, 1-s^2 A]] has
det(M)=1 and eigenvalues e^{+-i theta} with cos(theta) = 1 - s^2 A / 2, so the
scanned state collapses to a rank-2 modulated prefix sum:

    u_t   = s * Bu_t            (s folded into B on host)
    T1    = gamma*cos(t th) + sin(t th);  T2 = cos(t th) - gamma*sin(t th)
    E     = cumsum(T1 * u);     F = cumsum(T2 * u)
    x_t   = sin(t th) * E_t + cos(t th) * F_t
    gamma = (s - s^2 A / 2) / sin(theta)

Sharding: states P=256 split across 8 cores (32 each); inside a core, time
L=8192 is folded 4x into partitions -> tiles are (128=[4 chunks x 32 states],
2048).  Fold-chunk carries are fixed with per-partition offsets folded into
the demodulation bias; the offsets come from row sums that the modulation
computes for free (tensor_tensor_reduce accum), so they are ready before the
scans finish.  Each core emits a partial (H, L) bf16 output; the host sums
partials, adds input*D, and transposes - the unshard step.

Device-side structure per core (driven by measured HW behavior):
  - input arrives HOST-pretransposed in jt-major layout (no DMA transpose)
  - the DVE scan runs at 2 cycles/col regardless of dtype and DVE+Pool
    contend for SBUF ports, so ALL elementwise work runs on DVE (bf16 2x
    mode) and Pool stays idle; Act (separate ports) does psum evacuation
  - the 2048-col scans are split into two INDEPENDENT 1024-col scans
    (no chaining); the second chunk's missing prefix is folded into its
    demod bias (offs + first-chunk finals)
  - x = t1 + t2 is absorbed into the projection via PSUM accumulation;
    projection matmuls of neighboring fold-chunks are interleaved so their
    accumulation chains can overlap on the PE
  - all tables are host-precomputed fp64 -> bf16
"""

import numpy as np

L, H, P = 8192, 128, 256
NCORES = 8
SLOC = P // NCORES          # states per core
FOLD = 4                    # time chunks folded into partitions
CL = L // FOLD              # 2048 free columns per partition row
NPART = FOLD * SLOC         # 128
JT = 512                    # j-tile width (psum bank)
NJT = CL // JT              # 4
SCH = 1024                  # scan chunk (2 j-tiles)
CL2 = 2 * CL

_CACHE: dict = {}


def _build_bass(split_waits=True):
    import concourse.bass as bass
    import concourse.mybir as mybir
    import concourse.tile as tile

    dt = mybir.dt.float32
    bt = mybir.dt.bfloat16
    Alu = mybir.AluOpType

    nc = bass.Bass(
        trn_type="TRN2",
        target_bir_lowering=False,
        debug=False,
        num_devices=NCORES,
    )

    inpT_d = nc.dram_tensor("inpT", [NPART, L], bt, kind="ExternalInput").ap()
    Bt_d = nc.dram_tensor("Bt", [H, 2 * SLOC], bt, kind="ExternalInput").ap()
    T1_d = nc.dram_tensor("T1", [NPART, CL], bt, kind="ExternalInput").ap()
    T2_d = nc.dram_tensor("T2", [NPART, CL], bt, kind="ExternalInput").ap()
    Sd_d = nc.dram_tensor("Sd", [NPART, CL], bt, kind="ExternalInput").ap()
    Cd_d = nc.dram_tensor("Cd", [NPART, CL], bt, kind="ExternalInput").ap()
    Ctr_d = nc.dram_tensor("Ctr", [NPART, H], bt, kind="ExternalInput").ap()
    Cti_d = nc.dram_tensor("Cti", [NPART, H], bt, kind="ExternalInput").ap()
    Wm_d = nc.dram_tensor("Wm", [NPART, NPART], bt, kind="ExternalInput").ap()
    outp = nc.dram_tensor("outp", [H, L], bt, kind="ExternalOutput").ap()

    with tile.TileContext(nc) as tc:
        cpool = tc.alloc_tile_pool(name="const", bufs=1)
        big = tc.alloc_tile_pool(name="big", bufs=1)
        evac = tc.alloc_tile_pool(name="evac", bufs=2)
        osbp = tc.alloc_tile_pool(name="osbp", bufs=2)
        psum_bu = tc.alloc_tile_pool(name="psum_bu", bufs=2, space="PSUM")
        psum_o = tc.alloc_tile_pool(name="psum_o", bufs=2, space="PSUM")

        # DMA priority order: Bu weights, first input block, mod tables, rest
        Bt = cpool.tile_from(Bt_d)
        inpT = big.tile([NPART, L], bt, tag="inpT")
        nc.sync.dma_start(out=inpT[:, 0:2048], in_=inpT_d[:, 0:2048])
        T1 = big.tile_from(T1_d)
        nc.sync.dma_start(out=inpT[:, 2048:4096], in_=inpT_d[:, 2048:4096])
        T2 = big.tile_from(T2_d)
        nc.sync.dma_start(out=inpT[:, 4096:6144], in_=inpT_d[:, 4096:6144])
        nc.sync.dma_start(out=inpT[:, 6144:8192], in_=inpT_d[:, 6144:8192])
        Sd = big.tile_from(Sd_d)
        Cd = big.tile_from(Cd_d)
        Ctr = cpool.tile_from(Ctr_d)
        Cti = cpool.tile_from(Cti_d)
        Wm = cpool.tile_from(Wm_d)

        ones = cpool.tile([NPART, SCH], bt)
        nc.gpsimd.memset(ones[:], 1.0)

        acc = cpool.tile([NPART, 16], dt)
        Y1 = big.tile([NPART, CL2], bt, tag="Y1")   # (T1*u_r | T1*u_i)
        Y2 = big.tile([NPART, CL2], bt, tag="Y2")   # (T2*u_r | T2*u_i)
        E1 = big.tile([NPART, CL2], bt, tag="E1")   # (Er | Ei)
        E2 = big.tile([NPART, CL2], bt, tag="E2")   # (Fr | Fi)

        def scan(arr, y, s):
            bass.BassGpSimd.tensor_tensor_scan(
                nc.vector, arr[:, s], ones[:], y[:, s], 0.0, Alu.mult, Alu.add
            )

        # ---- Bu matmuls + modulation (rowsum accum fused) + scans ch0 ----
        for jt in range(NJT):
            pbu = psum_bu.tile([NPART, 2 * JT], dt, tag="bu")
            for c in range(FOLD):
                rhs = inpT[:, jt * CL + c * JT : jt * CL + (c + 1) * JT]
                ps = slice(c * SLOC, (c + 1) * SLOC)
                nc.tensor.matmul(
                    pbu[ps, 0:JT], Bt[:, 0:SLOC], rhs, start=True, stop=True,
                    tile_position=(0, c * SLOC),
                )
                nc.tensor.matmul(
                    pbu[ps, JT : 2 * JT], Bt[:, SLOC : 2 * SLOC], rhs,
                    start=True, stop=True,
                    tile_position=(0, c * SLOC),
                )
            U = evac.tile([NPART, 2 * JT], bt, tag="U")
            nc.scalar.copy(U[:], pbu[:])
            js = slice(jt * JT, (jt + 1) * JT)
            jsi = slice(CL + jt * JT, CL + (jt + 1) * JT)
            a = jt * 4
            for i, (dst, sl, tab) in enumerate((
                (Y1, js, T1), (Y2, js, T2), (Y1, jsi, T1), (Y2, jsi, T2),
            )):
                u = U[:, 0:JT] if i < 2 else U[:, JT : 2 * JT]
                nc.vector.scalar_tensor_tensor(
                    dst[:, sl], u, 1.0, tab[:, js], Alu.mult, Alu.mult,
                    accum_out=acc[:, a + i : a + i + 1],
                )
            if jt == 1:
                for arr, y in ((E1, Y1), (E2, Y2)):
                    scan(arr, y, slice(0, SCH))
                    scan(arr, y, slice(CL, CL + SCH))

        # ---- carry offsets from the accumulated row sums ----
        s8 = cpool.tile([NPART, 8], dt)
        fins = cpool.tile([NPART, 4], bt)
        nc.vector.tensor_add(s8[:], acc[:, 0:8], acc[:, 8:16])
        nc.vector.tensor_add(fins[:], s8[:, 0:4], s8[:, 4:8])
        poff = psum_bu.tile([NPART, 4], dt, tag="bu")
        nc.tensor.matmul(poff[:], Wm[:], fins[:], start=True, stop=True)
        offs = cpool.tile([NPART, 4], dt)
        nc.scalar.copy(offs[:], poff[:])
        # second-chunk bias = offs + first-chunk finals (scans are unchained)
        f0 = cpool.tile([NPART, 4], dt)
        for i, (arr, col) in enumerate(
            ((E1, SCH), (E2, SCH), (E1, CL + SCH), (E2, CL + SCH))
        ):
            nc.scalar.copy(f0[:, i : i + 1], arr[:, col - 1 : col])
        offs2 = cpool.tile([NPART, 4], dt)

        # ---- demod ch0 -> proj ch0 -> scans ch1 -> demod ch1 -> proj ch1 --
        t1 = big.tile([NPART, CL2], bt, tag="t1")
        t2 = big.tile([NPART, CL2], bt, tag="t2")

        def demod(ch, off):
            sc = slice(ch * SCH, (ch + 1) * SCH)
            sci = slice(CL + ch * SCH, CL + (ch + 1) * SCH)
            nc.vector.scalar_tensor_tensor(
                t1[:, sc], E1[:, sc], off[:, 0:1], Sd[:, sc], Alu.add, Alu.mult
            )
            nc.vector.scalar_tensor_tensor(
                t2[:, sc], E2[:, sc], off[:, 1:2], Cd[:, sc], Alu.add, Alu.mult
            )
            nc.vector.scalar_tensor_tensor(
                t1[:, sci], E1[:, sci], off[:, 2:3], Sd[:, sc], Alu.add, Alu.mult
            )
            nc.vector.scalar_tensor_tensor(
                t2[:, sci], E2[:, sci], off[:, 3:4], Cd[:, sc], Alu.add, Alu.mult
            )

        def proj(ch):
            for cpair in ((0, 1), (2, 3)):
                pos = {}
                for c in cpair:
                    po = psum_o.tile([NPART, 2 * JT], dt, tag="out")
                    pos[c] = po
                for half in range(2):
                    js = slice((2 * ch + half) * JT, (2 * ch + half + 1) * JT)
                    jsi = slice(CL + js.start, CL + js.stop)
                    pd = slice(half * JT, (half + 1) * JT)
                    for k, (mat, sl) in enumerate((
                        (Ctr, js), (Ctr, None), (Cti, jsi), (Cti, None),
                    )):
                        src = t1 if k % 2 == 0 else t2
                        msl = js if k < 2 else jsi
                        for c in cpair:
                            ps = slice(c * SLOC, (c + 1) * SLOC)
                            nc.tensor.matmul(
                                pos[c][:, pd], mat[ps, :], src[ps, msl],
                                start=(k == 0),
                                stop=(k == 3),
                                tile_position=(c * SLOC, 0),
                            )
                for c in cpair:
                    osb = osbp.tile([NPART, 2 * JT], bt, tag="osb")
                    nc.scalar.copy(osb[:], pos[c][:])
                    nc.sync.dma_start(
                        out=outp[:, c * CL + 2 * ch * JT : c * CL + (2 * ch + 2) * JT],
                        in_=osb[:],
                    )

        demod(0, offs)
        proj(0)
        for arr, y in ((E1, Y1), (E2, Y2)):
            scan(arr, y, slice(SCH, CL))
            scan(arr, y, slice(CL + SCH, CL2))
        nc.vector.tensor_add(offs2[:], offs[:], f0[:])
        demod(1, offs2)
        proj(1)

        for p in (psum_o, psum_bu, osbp, evac, big, cpool):
            p.release()
    if split_waits:
        _split_matmul_waits(nc, mybir)
    return nc


def _split_matmul_waits(nc, mybir):
    """Hardware instruction structs fit a limited number of embedded sync
    waits (1 for the fp32 self-loading LDWEIGHTS matmul, 2 for ACT/DVE/POOL
    compute structs); move extra waits onto an inserted same-queue no-op."""
    caps = {"InstMatmult": 1}
    skip = {"InstNoOp", "InstAllEngineBarrier", "InstSync"}
    k = 0
    for bb in nc.main_func.blocks:
        insts = bb.instructions
        i = 0
        while i < len(insts):
            ins = insts[i]
            tn = type(ins).__name__
            if tn not in skip and ins.sync_info is not None:
                cap = caps.get(tn, 1)
                w = list(ins.sync_info.on_wait or [])
                if len(w) > cap:
                    for wj in w[:-cap]:
                        nop = mybir.InstNoOp(
                            name=f"I-mmdep-{k}",
                            engine=ins.engine,
                            ins=[],
                            outs=[],
                            sync_info=mybir.SyncInfo(
                                on_wait=[wj], on_update=[]
                            ),
                        )
                        k += 1
                        insts.insert(i, nop)
                        i += 1
                    ins.sync_info = mybir.SyncInfo(
                        on_wait=w[-cap:], on_update=ins.sync_info.on_update
                    )
            i += 1


def _host_prep(inputs):
    import ml_dtypes
    bf16 = ml_dtypes.bfloat16
    inp = np.asarray(inputs["input_sequence"], np.float32)
    # host pre-transpose into jt-major layout: SBUF col jt*CL + c*JT + j
    # holds natural time t = c*CL + jt*JT + j
    inpT = np.ascontiguousarray(
        inp.T.reshape(H, FOLD, NJT, JT).transpose(0, 2, 1, 3).reshape(H, L)
    ).astype(bf16)
    A = np.maximum(np.asarray(inputs["A_diag_raw"], np.float64), 0.0)
    s = 1.0 / (1.0 + np.exp(-np.asarray(inputs["steps_raw"], np.float64)))
    Br = np.asarray(inputs["B_real"], np.float64)
    Bi = np.asarray(inputs["B_img"], np.float64)
    Cr = np.asarray(inputs["C_real"], np.float64)
    Ci = np.asarray(inputs["C_img"], np.float64)

    costh = 1.0 - s * s * A / 2.0
    sinth = np.sqrt(np.maximum(1.0 - costh * costh, 1e-300))
    theta = np.arctan2(sinth, costh)
    gamma = (s - s * s * A / 2.0) / sinth

    twopi = 2.0 * np.pi
    q = np.arange(NPART)
    Wm_f = ((q[:, None] % SLOC == q[None, :] % SLOC)
            & (q[:, None] // SLOC < q[None, :] // SLOC)).astype(bf16)
    tbase = np.repeat(np.arange(FOLD) * CL, SLOC).astype(np.float64)
    j = np.arange(CL, dtype=np.float64)

    in_maps = []
    for k in range(NCORES):
        sl = slice(k * SLOC, (k + 1) * SLOC)
        Bt = np.empty((H, 2 * SLOC), bf16)
        Bt[:, 0:SLOC] = (s[sl, None] * Br[sl]).T.astype(bf16)
        Bt[:, SLOC:] = (s[sl, None] * Bi[sl]).T.astype(bf16)
        Ctr = np.tile(Cr[:, sl].T, (FOLD, 1)).astype(bf16)
        Cti = np.tile(-Ci[:, sl].T, (FOLD, 1)).astype(bf16)

        th_q = np.tile(theta[sl], FOLD)          # (NPART,)
        g_q = np.tile(gamma[sl], FOLD)[:, None]  # (NPART, 1)
        ang = np.mod((tbase[:, None] + j[None, :]) * th_q[:, None], twopi)
        sinT = np.sin(ang)
        cosT = np.cos(ang)
        in_maps.append({
            "inpT": inpT,
            "Bt": Bt,
            "T1": (g_q * cosT + sinT).astype(bf16),
            "T2": (cosT - g_q * sinT).astype(bf16),
            "Sd": sinT.astype(bf16),
            "Cd": cosT.astype(bf16),
            "Ctr": Ctr,
            "Cti": Cti,
            "Wm": Wm_f,
        })
    return in_maps


LAST_RESULTS = None


def kernel(**inputs) -> np.ndarray:
    global LAST_RESULTS
    from concourse.bass_utils import run_bass_kernel_spmd

    if "nc" not in _CACHE:
        _CACHE["nc"] = _build_bass()
    nc = _CACHE["nc"]

    in_maps = _host_prep(inputs)
    res = run_bass_kernel_spmd(nc, in_maps, core_ids=list(range(NCORES)))
    LAST_RESULTS = res
    part = np.zeros((H, L), np.float32)
    for r in res.results:
        part += r["outp"].astype(np.float32)
    out = part.T + np.asarray(inputs["input_sequence"], np.float32) * np.asarray(
        inputs["D"], np.float32
    )
    return np.ascontiguousarray(out)
